# revision 21
# baseline (speedup 1.0000x reference)
"""Causal self-attention on 8 TRN2 NeuronCores, fp8-DoubleRow edition.

Reference computation (B=4, T=2048, C=1024, H=16, D=64, fp32):
    qkv = x @ W_attn + b_attn ; split q,k,v ; per-head causal softmax(q k^T / 8) @ v
    y = heads @ W_proj + b_proj

Sharding: core c handles batch b = c//2 and head-half hh = c%2 (8 heads).
QKV weights column-split, W_proj row-split; host sums the two partial
projections per batch and adds b_proj + the folded v-bias term.

Speed design (cost-model driven):
  - All GEMMs except the output projection run in fp8e4 with
    perf_mode=DoubleRow (K=2x128 per matmul, 0.5 cycles/row): QKV,
    S^T = K^T Q (via [64, 2, T] operand layout with j=1 zero-padded),
    and A@V (kc-chunk pairs in the j dim).  W_q/W_k/W_v are host-scaled
    by 32 so fp8 quantization stays in range; the q.k logit scale is
    folded into the exp scale (2^-13), and the A@V denominator ones
    column is 32.0 so the v-side 32x cancels in normalization.
  - QKV biases are folded into the GEMM as a 5th DoubleRow matmul
    (stationary = bias row at contraction slot (p=0, j=0), moving = a
    constant ones tile), so all psum->SBUF drains are pure copies that
    can be placed on either ACT or DVE.
  - softmax exp splits across engines: full (below-diagonal) chunks run
    true exp on ACT writing fp8e4 directly; diagonal chunks run on DVE
    as one fused op: u8 = saturate(psum * log2(e)/1024 + maskbias),
    where maskbias is +56 (valid) / -30000 (masked).  The u8 integer IS
    the e4m3 bit pattern 2^((u8-56)/8), i.e. a piecewise-linear exp2
    approximation of e^logit; softmax averaging suppresses the mantissa
    approximation error.  Saturation at 0 implements the causal mask.
  - A@V accumulates [65, q] per head in one [128, 1024] psum tile (both
    heads side by side); row 64 is the 32.0-ones denominator.  One DVE
    reciprocal [1, 1024] -> Pool partition_broadcast -> two DVE
    multiplies write normalized bf16 yT for the bf16 projection.
  - Emission order doubles as per-engine program order: QKV groups of
    the next t-quarter and projection chunks interleave between
    attention units to keep the PE dense (p-state) while ACT/DVE grind.
"""

import numpy as np
import ml_dtypes

import concourse.bacc as bacc
import concourse.mybir as mybir
import concourse.tile as tile
from concourse import library_config
from concourse.bass_utils import run_bass_kernel_spmd

F32 = mybir.dt.float32
BF16 = mybir.dt.bfloat16
U8 = mybir.dt.uint8
F8E4 = mybir.dt.float8e4
AF = mybir.ActivationFunctionType
ALU = mybir.AluOpType
DR = mybir.MatmulPerfMode.DoubleRow

N_CORES = 8
B, T, C = 4, 2048, 1024
H, D = 16, 64
CH = 512            # features per core (8 heads * 64)
NFO = 4             # head-pair chunks of 128 features
NTQ = 4             # t quarters of 512
WSC = 32.0          # host weight scale folded into fp8 quantization
K8 = float(np.log2(np.e) / 2048.0)   # psum -> e4m3-exp2 bits slope (2x: stride-0 j)
EXPB = 56.0                          # e4m3 bits bias: 2^((u8-56)/8)
ACT_SCALE = float(2.0 ** -14)        # psum -> logit scale for true exp (2x: stride-0 j)

_cached = {}


def _build_nc():
    nc = bacc.Bacc("TRN2", debug=False, num_devices=N_CORES)

    d_x8 = nc.dram_tensor("x8", [C, T], U8, kind="ExternalInput")
    d_wq = nc.dram_tensor("wq8", [C, CH], U8, kind="ExternalInput")
    d_wk = nc.dram_tensor("wk8", [C, CH], U8, kind="ExternalInput")
    d_wv = nc.dram_tensor("wv8", [C, CH], U8, kind="ExternalInput")
    d_bqk = nc.dram_tensor("bqk8", [128, 2, 1024], U8, kind="ExternalInput")
    d_ones = nc.dram_tensor("ones8", [128, 2, 512], U8, kind="ExternalInput")
    d_masks = nc.dram_tensor("masks", [128, 2, 512], BF16, kind="ExternalInput")
    d_wp = nc.dram_tensor("wp", [CH, C], BF16, kind="ExternalInput")
    d_xb = nc.dram_tensor("xb", [C, 512], BF16, kind="ExternalInput")
    d_wvb = nc.dram_tensor("wvb", [C, CH], BF16, kind="ExternalInput")
    d_m01 = nc.dram_tensor("m01", [128, 2, 128], BF16, kind="ExternalInput")
    d_out = nc.dram_tensor("out", [T, C], BF16, kind="ExternalOutput")

    with tile.TileContext(nc) as tc, nc.allow_low_precision(
        reason="fp8 staging; fp32 PSUM accumulation; bf16 projection"
    ), (
        tc.tile_pool(name="persist", bufs=1)
    ) as persist, (
        tc.tile_pool(name="pW", bufs=1)
    ) as pW, (
        tc.tile_pool(name="pX", bufs=1)
    ) as pX, (
        tc.tile_pool(name="pO", bufs=3)
    ) as pO, (
        tc.tile_pool(name="p2e", bufs=20)
    ) as p2e, (
        tc.tile_pool(name="p2b", bufs=9)
    ) as p2b, (
        tc.tile_pool(name="p2r", bufs=2)
    ) as p2r, (
        tc.tile_pool(name="psA", bufs=2, space="PSUM")
    ) as psA, (
        tc.tile_pool(name="psS", bufs=2, space="PSUM")
    ) as psS, (
        tc.tile_pool(name="psY", bufs=1, space="PSUM")
    ) as psY:
        # persistent on-chip tensors
        qT8 = [persist.tile([128, T], U8, tag=f"qT{fo}", name=f"qT{fo}") for fo in range(NFO)]
        kT8 = [persist.tile([128, T], U8, tag=f"kT{fo}", name=f"kT{fo}") for fo in range(NFO)]
        v8 = [persist.tile([128, 2, 8, 80], U8, tag=f"v{i}", name=f"v{i}") for i in range(8)]
        yT = [persist.tile([128, T], BF16, tag=f"yT{fo}", name=f"yT{fo}") for fo in range(NFO)]
        bqk_sb = persist.tile([128, 2, 1024], U8, tag="bqk")
        ones_sb = persist.tile([128, 2, 512], U8, tag="ones")
        masks_sb = persist.tile([128, 2, 512], BF16, tag="masks")
        wq_sb = pW.tile([128, 8, CH], U8, tag="wq")
        wk_sb = pW.tile([128, 8, CH], U8, tag="wk")
        wv_sb = pW.tile([128, 8, CH], U8, tag="wv")
        wp_sb = pW.tile([128, 4, C], BF16, tag="wp")
        x_tiles = [pX.tile([128, 8, 512], U8, tag=f"x{tq}", name=f"x{tq}") for tq in range(NTQ)]
        xb_sb = pX.tile([128, 8, 512], BF16, tag="xb")
        wvb_sb = pW.tile([128, 8, CH], BF16, tag="wvb")
        vb = [persist.tile([128, 8, 80], BF16, tag=f"vb{i}", name=f"vb{i}") for i in range(4)]
        m01_sb = persist.tile([128, 2, 128], BF16, tag="m01")

        nc.gpsimd.load_library(library_config.attn)

        def _w_piece(dst, src, c0, c1):
            nc.sync.dma_start(
                dst[:, c0:c1, :],
                src.ap()[128 * c0 : 128 * c1, :].rearrange("(c p) f -> p c f", p=128),
            )

        nc.sync.dma_start(
            x_tiles[0][:, 0:2, :],
            d_x8.ap()[0:256, 0:512].rearrange("(c p) t -> p c t", p=128),
        )
        _w_piece(wq_sb, d_wq, 0, 2)
        nc.sync.dma_start(bqk_sb[:], d_bqk.ap())
        nc.sync.dma_start(ones_sb[:], d_ones.ap())
        nc.sync.dma_start(
            x_tiles[0][:, 2:8, :],
            d_x8.ap()[256:1024, 0:512].rearrange("(c p) t -> p c t", p=128),
        )
        _w_piece(wq_sb, d_wq, 2, 8)
        _w_piece(wk_sb, d_wk, 0, 8)
        nc.sync.dma_start(xb_sb[:], d_xb.ap().rearrange("(c p) t -> p c t", p=128))
        nc.sync.dma_start(wvb_sb[:], d_wvb.ap().rearrange("(c p) f -> p c f", p=128))
        nc.sync.dma_start(masks_sb[:], d_masks.ap())
        nc.sync.dma_start(m01_sb[:], d_m01.ap())
        _w_piece(wv_sb, d_wv, 0, 8)
        for tq in range(1, NTQ):
            nc.sync.dma_start(
                x_tiles[tq][:],
                d_x8.ap()[:, 512 * tq : 512 * (tq + 1)].rearrange("(c p) t -> p c t", p=128),
            )
        nc.sync.dma_start(wp_sb[:], d_wp.ap().rearrange("(c p) f -> p c f", p=128))

        # one-time fills (Pool): v ones-columns carry the denominator
        for i in range(8):
            # 0x60 is the e4m3 bit pattern of 32.0 (the denominator scale)
            nc.gpsimd.memset(v8[i][:, :, :, 64:65], 96.0)
        for i in range(4):
            nc.gpsimd.memset(vb[i][:, :, 64:65], 32.0)

        def _j2(ap2d):
            # [64, w] -> [64, 2, w] with a stride-0 j dim: both DoubleRow
            # k-tiles read the same data, so S comes out 2x (folded into the
            # exp scale).  Avoids materializing zeroed j=1 operand planes.
            p, w = ap2d.shape
            return ap2d.rearrange("p (one w) -> p one w", one=1).broadcast_to([p, 2, w])

        # flexible psum->SBUF drains: route to ACT or DVE
        def flex_copy(eng, out_ap, in_ap):
            if eng == "A":
                nc.scalar.copy(out_ap, in_ap)
            else:
                nc.vector.tensor_copy(out_ap, in_ap)

        def emit_qk_group(tq, w_sb, boff, dst, fo, eng):
            ps = psA.tile([128, 512], F32, tag="psA", name="ps_qk")
            for i in range(4):
                nc.tensor.matmul(
                    ps[:],
                    w_sb[:, 2 * i : 2 * i + 2, 128 * fo : 128 * (fo + 1)].bitcast(F8E4),
                    x_tiles[tq][:, 2 * i : 2 * i + 2, :].bitcast(F8E4),
                    start=(i == 0),
                    stop=False,
                    perf_mode=DR,
                )
            nc.tensor.matmul(
                ps[:],
                bqk_sb[:, :, boff + 128 * fo : boff + 128 * (fo + 1)].bitcast(F8E4),
                ones_sb[:].bitcast(F8E4),
                start=False,
                stop=True,
                perf_mode=DR,
            )
            flex_copy(eng, dst[fo][:, 512 * tq : 512 * (tq + 1)].bitcast(F8E4), ps[:])

        def emit_v_group(tq, ts, eng):
            tci = 4 * tq + ts
            ps = psA.tile([128, 512], F32, tag="psA", name="ps_v")
            if tq == 0:
                # first quarter: bf16 GEMM so early (small-n) rows keep a
                # precise v; feeds both the bf16 and the fp8 A@V paths
                for i in range(8):
                    nc.tensor.matmul(
                        ps[:],
                        xb_sb[:, i, 128 * ts : 128 * (ts + 1)],
                        wvb_sb[:, i, :],
                        start=(i == 0),
                        stop=(i == 7),
                    )
                flex_copy(
                    "A",
                    vb[ts][:, :, 0:64],
                    ps[:].rearrange("p (h d) -> p h d", h=8),
                )
            else:
                for i in range(4):
                    nc.tensor.matmul(
                        ps[:],
                        x_tiles[tq][:, 2 * i : 2 * i + 2, 128 * ts : 128 * (ts + 1)].bitcast(F8E4),
                        wv_sb[:, 2 * i : 2 * i + 2, :].bitcast(F8E4),
                        start=(i == 0),
                        stop=(i == 3),
                        perf_mode=DR,
                    )
            flex_copy(
                eng,
                v8[tci // 2][:, tci % 2, :, 0:64].bitcast(F8E4),
                ps[:].rearrange("p (h d) -> p h d", h=8),
            )

        def emit_qkv_group(tq, g):
            if g < 4:
                emit_qk_group(tq, wq_sb, 0, qT8, g, "D" if (tq == 3 or g % 2 == 1) else "A")
            elif g < 8:
                emit_qk_group(tq, wk_sb, 512, kT8, g - 4, "D" if (tq == 3 or g % 2 == 0) else "A")
            else:
                emit_v_group(tq, g - 8, "D" if tq == 3 else "A")

        def emit_proj_tc(tci):
            o_sb = pO.tile([128, C], BF16, tag="o", name="o_sb")
            for co in range(2):
                ps = psA.tile([128, 512], F32, tag="psA", name="ps_o")
                for fo in range(NFO):
                    nc.tensor.matmul(
                        ps[:],
                        yT[fo][:, 128 * tci : 128 * (tci + 1)],
                        wp_sb[:, fo, 512 * co : 512 * (co + 1)],
                        start=(fo == 0),
                        stop=(fo == 3),
                    )
                flex_copy("A" if (co == 0 or tci < 4) else "D", o_sb[:, 512 * co : 512 * (co + 1)], ps[:])
                nc.sync.dma_start(
                    d_out.ap()[128 * tci : 128 * (tci + 1), 512 * co : 512 * (co + 1)],
                    o_sb[:, 512 * co : 512 * (co + 1)],
                )

        full_ctr = [0]

        def sexp_attn0(fo, after_first=None, fillers=None):
            # b = 0: all-diagonal unit on the bf16 path (true exp, bf16 A@V)
            ebs = []
            for c in range(4):
                qoff = 128 * c
                pS = psS.tile([128, 1024], F32, tag="pS", name="pS")
                for hb in (0, 64):
                    nc.tensor.matmul(
                        pS[:, (0 if hb == 0 else 512) + qoff : (512 if hb == 0 else 1024)],
                        _j2(kT8[fo][hb : hb + 64, 128 * c : 128 * (c + 1)]).bitcast(F8E4),
                        _j2(qT8[fo][hb : hb + 64, qoff:512]).bitcast(F8E4),
                        perf_mode=DR,
                    )
                eb = p2b.tile([128, 1024], BF16, tag="eb", name="eb")
                if c > 0:
                    nc.gpsimd.memset(
                        eb[:].rearrange("p (h w) -> p h w", h=2)[:, :, 0:qoff], 0.0
                    )
                nc.scalar.activation(
                    eb[:].rearrange("p (h w) -> p h w", h=2)[:, :, qoff:512],
                    pS[:].rearrange("p (h w) -> p h w", h=2)[:, :, qoff:512],
                    AF.Exp,
                    scale=ACT_SCALE,
                )
                nc.vector.tensor_tensor(
                    eb[:].rearrange("p (h w) -> p h w", h=2)[:, :, qoff : qoff + 128],
                    eb[:].rearrange("p (h w) -> p h w", h=2)[:, :, qoff : qoff + 128],
                    m01_sb[:],
                    op=ALU.mult,
                )
                ebs.append(eb)
                if c == 0 and after_first is not None:
                    after_first()
                elif fillers:
                    fillers.pop(0)()
            return ebs

        def av_attn0(fo, ebs):
            py = psY.tile([128, 1024], F32, tag="py", name="py")
            for c in range(4):
                for hb, h in ((0, 2 * fo), (64, 2 * fo + 1)):
                    col = 0 if hb == 0 else 512
                    nc.tensor.matmul(
                        py[0:65, col : col + 512],
                        vb[c][:, h, 0:65],
                        ebs[c][:, col : col + 512],
                        start=(c == 0),
                        stop=(c == 3),
                    )
            emit_norm(fo, 0, py)

        def emit_norm(fo, b, py):
            # per-head chain: small ops pipeline across DVE/Pool, shortening
            # the py-buffer critical path (psY has a single buffer)
            q0 = 512 * b
            recA = p2r.tile([1, 512], F32, tag="recA", name="recA")
            nc.vector.reciprocal(recA[:], py[64:65, 0:512])
            bcA = p2r.tile([64, 512], F32, tag="bcA", name="bcA")
            nc.gpsimd.partition_broadcast(bcA[:], recA[:])
            recB = p2r.tile([1, 512], F32, tag="recB", name="recB")
            nc.vector.reciprocal(recB[:], py[64:65, 512:1024])
            bcB = p2r.tile([64, 512], F32, tag="bcB", name="bcB")
            nc.gpsimd.partition_broadcast(bcB[:], recB[:])
            nc.vector.tensor_tensor(
                yT[fo][0:64, q0 : q0 + 512], py[0:64, 0:512], bcA[:], op=ALU.mult
            )
            nc.vector.tensor_tensor(
                yT[fo][64:128, q0 : q0 + 512], py[0:64, 512:1024], bcB[:], op=ALU.mult
            )

        def sexp_attn(fo, b, after_first=None, fillers=None):
            if b == 0:
                return sexp_attn0(fo, after_first, fillers)
            q0 = 512 * b
            # chunk pairs in A@V DoubleRow j-dim.  diag pair 0 first (its
            # full-width A@V start matmul zeroes the psum region); then
            # interleave full pairs (ACT exp) with diag pair 1 (DVE exp) so
            # neither engine starves during the unit.
            fulls = [(2 * j, 0) for j in range(2 * b)]
            pairs = [(4 * b, 0)] + fulls[0:1] + [(4 * b + 2, 256)] + fulls[1:]
            ests = []
            for pi, (c0, av_qoff) in enumerate(pairs):
                eST = p2e.tile([128, 2, 1024], U8, tag="eST", name="eST")
                diag = c0 >= 4 * b
                if diag:
                    # zero the columns of this pair that no chunk writes
                    i0 = c0 - 4 * b
                    if i0 == 0:
                        nc.gpsimd.memset(
                            eST[:, 1, :].rearrange("p (h w) -> p h w", h=2)[:, :, 0:128],
                            0,
                        )
                    else:
                        nc.gpsimd.memset(
                            eST[:, 1, :].rearrange("p (h w) -> p h w", h=2)[:, :, 256:384],
                            0,
                        )
                for c in (c0, c0 + 1):
                    i = c - 4 * b
                    qoff = 128 * i if diag else 0
                    w = 512 - qoff
                    pS = psS.tile([128, 1024], F32, tag="pS", name="pS")
                    for hb in (0, 64):
                        nc.tensor.matmul(
                            pS[:, (0 if hb == 0 else 512) + qoff : (512 if hb == 0 else 1024)],
                            _j2(kT8[fo][hb : hb + 64, 128 * c : 128 * (c + 1)]).bitcast(F8E4),
                            _j2(qT8[fo][hb : hb + 64, q0 + qoff : q0 + 512]).bitcast(F8E4),
                            perf_mode=DR,
                        )
                    if diag:
                        # fused exp2-bits + causal mask + fp8 quantize (DVE)
                        nc.vector.scalar_tensor_tensor(
                            eST[:, c % 2, :].rearrange("p (h w) -> p h w", h=2)[:, :, qoff:512],
                            pS[:].rearrange("p (h w) -> p h w", h=2)[:, :, qoff:512],
                            K8,
                            masks_sb[:, :, 0:w],
                            op0=ALU.mult,
                            op1=ALU.add,
                        )
                    else:
                        # full chunk: true exp on ACT; in the ACT-hot rows a
                        # fraction goes to DVE instead
                        if (b == 3 and full_ctr[0] % 6 == 2) or (
                            b == 2 and full_ctr[0] % 3 == 1
                        ):
                            nc.vector.tensor_scalar(
                                eST[:, c % 2, :], pS[:], K8, EXPB, op0=ALU.mult, op1=ALU.add
                            )
                        else:
                            nc.scalar.activation(
                                eST[:, c % 2, :].bitcast(F8E4), pS[:], AF.Exp, scale=ACT_SCALE
                            )
                        full_ctr[0] += 1
                ests.append((c0, av_qoff, eST))
                if pi == 0 and after_first is not None:
                    after_first()
                elif fillers:
                    fillers.pop(0)()
            return ests

        def av_attn(fo, b, ests):
            if b == 0:
                av_attn0(fo, ests)
                return
            py = psY.tile([128, 1024], F32, tag="py", name="py")
            for pi, (c0, av_qoff, eST) in enumerate(ests):
                first = pi == 0
                last = pi == len(ests) - 1
                for hb, h in ((0, 2 * fo), (64, 2 * fo + 1)):
                    col = 0 if hb == 0 else 512
                    nc.tensor.matmul(
                        py[0:65, col + av_qoff : col + 512],
                        v8[c0 // 2][:, :, h, 0:65].bitcast(F8E4),
                        eST[:, :, col + av_qoff : col + 512].bitcast(F8E4),
                        start=first,
                        stop=last,
                        perf_mode=DR,
                    )
            emit_norm(fo, b, py)

        for g in range(12):
            emit_qkv_group(0, g)
        from collections import deque

        pend = deque()
        for b in range(NTQ):
            for fo in range(NFO):
                cb = (lambda: av_attn(*pend.popleft())) if pend else None
                fill = []
                if b < NTQ - 1:
                    for g in range(3 * fo, 3 * fo + 3):
                        fill.append(lambda tq=b + 1, gg=g: emit_qkv_group(tq, gg))
                if b == 1:
                    fill.append(lambda t=fo: emit_proj_tc(t))
                elif b == 3:
                    fill.append(lambda t=4 + fo: emit_proj_tc(t))
                    fill.append(lambda t=8 + fo: emit_proj_tc(t))
                st = sexp_attn(fo, b, after_first=cb, fillers=fill)
                pend.append((fo, b, st))
                for f in fill:
                    f()
        while pend:
            av_attn(*pend.popleft())
        for tci in range(12, 16):
            emit_proj_tc(tci)

    nc.compile()
    return nc


def _get_nc():
    if "nc" not in _cached:
        _cached["nc"] = _build_nc()
    return _cached["nc"]


def _e4m3(a):
    return np.asarray(a, np.float32).astype(ml_dtypes.float8_e4m3).view(np.uint8)


def kernel(x, W_attn, b_attn, W_proj, b_proj):
    x = np.asarray(x, np.float32)
    W_attn = np.asarray(W_attn, np.float32)
    b_attn = np.asarray(b_attn, np.float32)
    W_proj = np.asarray(W_proj, np.float32)
    b_proj = np.asarray(b_proj, np.float32)

    nc = _get_nc()
    p = np.arange(128)[:, None]
    j = np.arange(128)[None, :]
    tri = np.where(j >= p, EXPB, -30000.0).astype(np.float32)   # [128,128]
    mask1 = np.concatenate([tri, np.full((128, 384), EXPB, np.float32)], axis=1)
    masks = np.stack([mask1, mask1], axis=1).astype(ml_dtypes.bfloat16)  # [128,2,512]

    ones8 = np.zeros((128, 2, 512), np.uint8)
    ones8[0, 0, :] = _e4m3(1.0)
    tri01 = (j >= p).astype(np.float32)
    m01 = np.stack([tri01, tri01], axis=1).astype(ml_dtypes.bfloat16)  # [128,2,128]

    in_maps = []
    for c in range(N_CORES):
        b, hh = divmod(c, 2)
        sl = slice(CH * hh, CH * (hh + 1))
        bqk = np.zeros((128, 2, 1024), np.uint8)
        bqk[0, 0, 0:512] = _e4m3(WSC * b_attn[0:C][sl])
        bqk[0, 0, 512:1024] = _e4m3(WSC * b_attn[C : 2 * C][sl])
        in_maps.append(
            {
                "x8": _e4m3(np.ascontiguousarray(x[b].T)),
                "wq8": _e4m3(WSC * W_attn[:, 0:C][:, sl]),
                "wk8": _e4m3(WSC * W_attn[:, C : 2 * C][:, sl]),
                "wv8": _e4m3(WSC * W_attn[:, 2 * C : 3 * C][:, sl]),
                "bqk8": bqk,
                "ones8": ones8,
                "masks": masks.view(np.uint16),
                "wp": np.ascontiguousarray(
                    W_proj[sl, :].astype(ml_dtypes.bfloat16)
                ).view(np.uint16),
                "xb": np.ascontiguousarray(x[b].T[:, 0:512]).astype(ml_dtypes.bfloat16).view(np.uint16),
                "wvb": np.ascontiguousarray(WSC * W_attn[:, 2 * C : 3 * C][:, sl]).astype(ml_dtypes.bfloat16).view(np.uint16),
                "m01": m01.view(np.uint16),
            }
        )

    try:
        res = run_bass_kernel_spmd(nc, in_maps, core_ids=list(range(N_CORES)))
    except Exception:
        # transient NRT device wedges happen; one retry is usually enough
        res = run_bass_kernel_spmd(nc, in_maps, core_ids=list(range(N_CORES)))

    bv = b_attn[2 * C : 3 * C]
    const_bias = (bv @ W_proj + b_proj).astype(np.float32)  # [C]
    def _as_f32(a):
        a = np.asarray(a)
        if a.dtype == np.uint16:
            a = a.view(ml_dtypes.bfloat16)
        return a.astype(np.float32)

    out = np.empty((B, T, C), np.float32)
    for b in range(B):
        out[b] = (
            _as_f32(res.results[2 * b]["out"])
            + _as_f32(res.results[2 * b + 1]["out"])
            + const_bias
        )
    return out


# revision 22
# speedup vs baseline: 1.0264x; 1.0264x over previous
"""Causal self-attention on 8 TRN2 NeuronCores, fp8-DoubleRow edition.

Reference computation (B=4, T=2048, C=1024, H=16, D=64, fp32):
    qkv = x @ W_attn + b_attn ; split q,k,v ; per-head causal softmax(q k^T / 8) @ v
    y = heads @ W_proj + b_proj

Sharding: core c handles batch b = c//2 and head-half hh = c%2 (8 heads).
QKV weights column-split, W_proj row-split; host sums the two partial
projections per batch and adds b_proj + the folded v-bias term.

Speed design (cost-model driven):
  - All GEMMs except the output projection run in fp8e4 with
    perf_mode=DoubleRow (K=2x128 per matmul, 0.5 cycles/row): QKV,
    S^T = K^T Q (via [64, 2, T] operand layout with j=1 zero-padded),
    and A@V (kc-chunk pairs in the j dim).  W_q/W_k/W_v are host-scaled
    by 32 so fp8 quantization stays in range; the q.k logit scale is
    folded into the exp scale (2^-13), and the A@V denominator ones
    column is 32.0 so the v-side 32x cancels in normalization.
  - QKV biases are folded into the GEMM as a 5th DoubleRow matmul
    (stationary = bias row at contraction slot (p=0, j=0), moving = a
    constant ones tile), so all psum->SBUF drains are pure copies that
    can be placed on either ACT or DVE.
  - softmax exp splits across engines: full (below-diagonal) chunks run
    true exp on ACT writing fp8e4 directly; diagonal chunks run on DVE
    as one fused op: u8 = saturate(psum * log2(e)/1024 + maskbias),
    where maskbias is +56 (valid) / -30000 (masked).  The u8 integer IS
    the e4m3 bit pattern 2^((u8-56)/8), i.e. a piecewise-linear exp2
    approximation of e^logit; softmax averaging suppresses the mantissa
    approximation error.  Saturation at 0 implements the causal mask.
  - A@V accumulates [65, q] per head in one [128, 1024] psum tile (both
    heads side by side); row 64 is the 32.0-ones denominator.  One DVE
    reciprocal [1, 1024] -> Pool partition_broadcast -> two DVE
    multiplies write normalized bf16 yT for the bf16 projection.
  - Emission order doubles as per-engine program order: QKV groups of
    the next t-quarter and projection chunks interleave between
    attention units to keep the PE dense (p-state) while ACT/DVE grind.
"""

import numpy as np
import ml_dtypes

import concourse.bacc as bacc
import concourse.mybir as mybir
import concourse.tile as tile
from concourse import library_config
from concourse.bass_utils import run_bass_kernel_spmd

F32 = mybir.dt.float32
BF16 = mybir.dt.bfloat16
U8 = mybir.dt.uint8
F8E4 = mybir.dt.float8e4
AF = mybir.ActivationFunctionType
ALU = mybir.AluOpType
DR = mybir.MatmulPerfMode.DoubleRow

N_CORES = 8
B, T, C = 4, 2048, 1024
H, D = 16, 64
CH = 512            # features per core (8 heads * 64)
NFO = 4             # head-pair chunks of 128 features
NTQ = 4             # t quarters of 512
WSC = 32.0          # host weight scale folded into fp8 quantization
K8 = float(np.log2(np.e) / 2048.0)   # psum -> e4m3-exp2 bits slope (2x: stride-0 j)
EXPB = 56.0                          # e4m3 bits bias: 2^((u8-56)/8)
ACT_SCALE = float(2.0 ** -14)        # psum -> logit scale for true exp (2x: stride-0 j)

_cached = {}


def _build_nc():
    nc = bacc.Bacc("TRN2", debug=False, num_devices=N_CORES)

    d_x8 = nc.dram_tensor("x8", [C, T], U8, kind="ExternalInput")
    d_wq = nc.dram_tensor("wq8", [C, CH], U8, kind="ExternalInput")
    d_wk = nc.dram_tensor("wk8", [C, CH], U8, kind="ExternalInput")
    d_wv = nc.dram_tensor("wv8", [C, CH], U8, kind="ExternalInput")
    d_bqk = nc.dram_tensor("bqk8", [128, 2, 1024], U8, kind="ExternalInput")
    d_ones = nc.dram_tensor("ones8", [128, 2, 512], U8, kind="ExternalInput")
    d_masks = nc.dram_tensor("masks", [128, 2, 512], BF16, kind="ExternalInput")
    d_wp = nc.dram_tensor("wp", [CH, C], BF16, kind="ExternalInput")
    d_xb = nc.dram_tensor("xb", [C, 512], BF16, kind="ExternalInput")
    d_wvb = nc.dram_tensor("wvb", [C, CH], BF16, kind="ExternalInput")
    d_m01 = nc.dram_tensor("m01", [128, 2, 128], BF16, kind="ExternalInput")
    d_out = nc.dram_tensor("out", [T, C], BF16, kind="ExternalOutput")

    with tile.TileContext(nc) as tc, nc.allow_low_precision(
        reason="fp8 staging; fp32 PSUM accumulation; bf16 projection"
    ), (
        tc.tile_pool(name="persist", bufs=1)
    ) as persist, (
        tc.tile_pool(name="pW", bufs=1)
    ) as pW, (
        tc.tile_pool(name="pX", bufs=1)
    ) as pX, (
        tc.tile_pool(name="pO", bufs=3)
    ) as pO, (
        tc.tile_pool(name="p2e", bufs=20)
    ) as p2e, (
        tc.tile_pool(name="p2b", bufs=9)
    ) as p2b, (
        tc.tile_pool(name="p2r", bufs=2)
    ) as p2r, (
        tc.tile_pool(name="psA", bufs=2, space="PSUM")
    ) as psA, (
        tc.tile_pool(name="psS", bufs=2, space="PSUM")
    ) as psS, (
        tc.tile_pool(name="psY", bufs=1, space="PSUM")
    ) as psY:
        # persistent on-chip tensors
        qT8 = [persist.tile([128, T], U8, tag=f"qT{fo}", name=f"qT{fo}") for fo in range(NFO)]
        kT8 = [persist.tile([128, T], U8, tag=f"kT{fo}", name=f"kT{fo}") for fo in range(NFO)]
        v8 = [persist.tile([128, 2, 8, 80], U8, tag=f"v{i}", name=f"v{i}") for i in range(8)]
        yT = [persist.tile([128, T], BF16, tag=f"yT{fo}", name=f"yT{fo}") for fo in range(NFO)]
        bqk_sb = persist.tile([128, 2, 1024], U8, tag="bqk")
        ones_sb = persist.tile([128, 2, 512], U8, tag="ones")
        masks_sb = persist.tile([128, 2, 512], BF16, tag="masks")
        wq_sb = pW.tile([128, 8, CH], U8, tag="wq")
        wk_sb = pW.tile([128, 8, CH], U8, tag="wk")
        wv_sb = pW.tile([128, 8, CH], U8, tag="wv")
        wp_sb = pW.tile([128, 4, C], BF16, tag="wp")
        x_tiles = [pX.tile([128, 8, 512], U8, tag=f"x{tq}", name=f"x{tq}") for tq in range(NTQ)]
        xb_sb = pX.tile([128, 8, 512], BF16, tag="xb")
        wvb_sb = pW.tile([128, 8, CH], BF16, tag="wvb")
        vb = [persist.tile([128, 8, 80], BF16, tag=f"vb{i}", name=f"vb{i}") for i in range(4)]
        m01_sb = persist.tile([128, 2, 128], BF16, tag="m01")

        nc.gpsimd.load_library(library_config.attn)

        def _w_piece(dst, src, c0, c1):
            nc.sync.dma_start(
                dst[:, c0:c1, :],
                src.ap()[128 * c0 : 128 * c1, :].rearrange("(c p) f -> p c f", p=128),
            )

        nc.sync.dma_start(
            x_tiles[0][:, 0:2, :],
            d_x8.ap()[0:256, 0:512].rearrange("(c p) t -> p c t", p=128),
        )
        _w_piece(wq_sb, d_wq, 0, 2)
        nc.sync.dma_start(bqk_sb[:], d_bqk.ap())
        nc.sync.dma_start(ones_sb[:], d_ones.ap())
        nc.sync.dma_start(
            x_tiles[0][:, 2:8, :],
            d_x8.ap()[256:1024, 0:512].rearrange("(c p) t -> p c t", p=128),
        )
        _w_piece(wq_sb, d_wq, 2, 8)
        _w_piece(wk_sb, d_wk, 0, 8)
        nc.sync.dma_start(xb_sb[:], d_xb.ap().rearrange("(c p) t -> p c t", p=128))
        nc.sync.dma_start(wvb_sb[:], d_wvb.ap().rearrange("(c p) f -> p c f", p=128))
        nc.sync.dma_start(masks_sb[:], d_masks.ap())
        nc.sync.dma_start(m01_sb[:], d_m01.ap())
        _w_piece(wv_sb, d_wv, 0, 8)
        for tq in range(1, NTQ):
            nc.sync.dma_start(
                x_tiles[tq][:],
                d_x8.ap()[:, 512 * tq : 512 * (tq + 1)].rearrange("(c p) t -> p c t", p=128),
            )
        nc.sync.dma_start(wp_sb[:], d_wp.ap().rearrange("(c p) f -> p c f", p=128))

        # one-time fills (Pool): v ones-columns carry the denominator
        for i in range(8):
            # 0x60 is the e4m3 bit pattern of 32.0 (the denominator scale)
            nc.gpsimd.memset(v8[i][:, :, :, 64:65], 96.0)
        for i in range(4):
            nc.gpsimd.memset(vb[i][:, :, 64:65], 32.0)

        def _j2(ap2d):
            # [64, w] -> [64, 2, w] with a stride-0 j dim: both DoubleRow
            # k-tiles read the same data, so S comes out 2x (folded into the
            # exp scale).  Avoids materializing zeroed j=1 operand planes.
            p, w = ap2d.shape
            return ap2d.rearrange("p (one w) -> p one w", one=1).broadcast_to([p, 2, w])

        # flexible psum->SBUF drains: route to ACT or DVE
        def flex_copy(eng, out_ap, in_ap):
            if eng == "A":
                nc.scalar.copy(out_ap, in_ap)
            else:
                nc.vector.tensor_copy(out_ap, in_ap)

        def emit_qk_group(tq, w_sb, boff, dst, fo, eng):
            ps = psA.tile([128, 512], F32, tag="psA", name="ps_qk")
            for i in range(4):
                nc.tensor.matmul(
                    ps[:],
                    w_sb[:, 2 * i : 2 * i + 2, 128 * fo : 128 * (fo + 1)].bitcast(F8E4),
                    x_tiles[tq][:, 2 * i : 2 * i + 2, :].bitcast(F8E4),
                    start=(i == 0),
                    stop=False,
                    perf_mode=DR,
                )
            nc.tensor.matmul(
                ps[:],
                bqk_sb[:, :, boff + 128 * fo : boff + 128 * (fo + 1)].bitcast(F8E4),
                ones_sb[:].bitcast(F8E4),
                start=False,
                stop=True,
                perf_mode=DR,
            )
            flex_copy(eng, dst[fo][:, 512 * tq : 512 * (tq + 1)].bitcast(F8E4), ps[:])

        def emit_v_group(tq, ts, eng):
            tci = 4 * tq + ts
            ps = psA.tile([128, 512], F32, tag="psA", name="ps_v")
            if tq == 0:
                # first quarter: bf16 GEMM so early (small-n) rows keep a
                # precise v; feeds both the bf16 and the fp8 A@V paths
                for i in range(8):
                    nc.tensor.matmul(
                        ps[:],
                        xb_sb[:, i, 128 * ts : 128 * (ts + 1)],
                        wvb_sb[:, i, :],
                        start=(i == 0),
                        stop=(i == 7),
                    )
                flex_copy(
                    "A",
                    vb[ts][:, :, 0:64],
                    ps[:].rearrange("p (h d) -> p h d", h=8),
                )
            else:
                for i in range(4):
                    nc.tensor.matmul(
                        ps[:],
                        x_tiles[tq][:, 2 * i : 2 * i + 2, 128 * ts : 128 * (ts + 1)].bitcast(F8E4),
                        wv_sb[:, 2 * i : 2 * i + 2, :].bitcast(F8E4),
                        start=(i == 0),
                        stop=(i == 3),
                        perf_mode=DR,
                    )
            flex_copy(
                eng,
                v8[tci // 2][:, tci % 2, :, 0:64].bitcast(F8E4),
                ps[:].rearrange("p (h d) -> p h d", h=8),
            )

        def emit_qkv_group(tq, g):
            if g < 4:
                emit_qk_group(tq, wq_sb, 0, qT8, g, "D" if (tq == 3 or g % 2 == 1) else "A")
            elif g < 8:
                emit_qk_group(tq, wk_sb, 512, kT8, g - 4, "D" if (tq == 3 or g % 2 == 0) else "A")
            else:
                emit_v_group(tq, g - 8, "D" if tq == 3 else "A")

        def emit_proj_tc(tci):
            o_sb = pO.tile([128, C], BF16, tag="o", name="o_sb")
            for co in range(2):
                ps = psA.tile([128, 512], F32, tag="psA", name="ps_o")
                for fo in range(NFO):
                    nc.tensor.matmul(
                        ps[:],
                        yT[fo][:, 128 * tci : 128 * (tci + 1)],
                        wp_sb[:, fo, 512 * co : 512 * (co + 1)],
                        start=(fo == 0),
                        stop=(fo == 3),
                    )
                flex_copy("A" if (co == 0 or tci < 4) else "D", o_sb[:, 512 * co : 512 * (co + 1)], ps[:])
                nc.sync.dma_start(
                    d_out.ap()[128 * tci : 128 * (tci + 1), 512 * co : 512 * (co + 1)],
                    o_sb[:, 512 * co : 512 * (co + 1)],
                )

        full_ctr = [0]

        def sexp_attn0(fo, after_first=None):
            # b = 0: all-diagonal unit on the bf16 path (true exp, bf16 A@V)
            ebs = []
            for c in range(4):
                qoff = 128 * c
                pS = psS.tile([128, 1024], F32, tag="pS", name="pS")
                for hb in (0, 64):
                    nc.tensor.matmul(
                        pS[:, (0 if hb == 0 else 512) + qoff : (512 if hb == 0 else 1024)],
                        _j2(kT8[fo][hb : hb + 64, 128 * c : 128 * (c + 1)]).bitcast(F8E4),
                        _j2(qT8[fo][hb : hb + 64, qoff:512]).bitcast(F8E4),
                        perf_mode=DR,
                    )
                eb = p2b.tile([128, 1024], BF16, tag="eb", name="eb")
                if c > 0:
                    nc.gpsimd.memset(
                        eb[:].rearrange("p (h w) -> p h w", h=2)[:, :, 0:qoff], 0.0
                    )
                nc.scalar.activation(
                    eb[:].rearrange("p (h w) -> p h w", h=2)[:, :, qoff:512],
                    pS[:].rearrange("p (h w) -> p h w", h=2)[:, :, qoff:512],
                    AF.Exp,
                    scale=ACT_SCALE,
                )
                nc.vector.tensor_tensor(
                    eb[:].rearrange("p (h w) -> p h w", h=2)[:, :, qoff : qoff + 128],
                    eb[:].rearrange("p (h w) -> p h w", h=2)[:, :, qoff : qoff + 128],
                    m01_sb[:],
                    op=ALU.mult,
                )
                ebs.append(eb)
                if c == 0 and after_first is not None:
                    after_first()
            return ebs

        def av_attn0(fo, ebs):
            py = psY.tile([128, 1024], F32, tag="py", name="py")
            for c in range(4):
                for hb, h in ((0, 2 * fo), (64, 2 * fo + 1)):
                    col = 0 if hb == 0 else 512
                    nc.tensor.matmul(
                        py[0:65, col : col + 512],
                        vb[c][:, h, 0:65],
                        ebs[c][:, col : col + 512],
                        start=(c == 0),
                        stop=(c == 3),
                    )
            emit_norm(fo, 0, py)

        def emit_norm(fo, b, py):
            # per-head chain: small ops pipeline across DVE/Pool, shortening
            # the py-buffer critical path (psY has a single buffer)
            q0 = 512 * b
            recA = p2r.tile([1, 512], F32, tag="recA", name="recA")
            nc.vector.reciprocal(recA[:], py[64:65, 0:512])
            bcA = p2r.tile([64, 512], F32, tag="bcA", name="bcA")
            nc.gpsimd.partition_broadcast(bcA[:], recA[:])
            recB = p2r.tile([1, 512], F32, tag="recB", name="recB")
            nc.vector.reciprocal(recB[:], py[64:65, 512:1024])
            bcB = p2r.tile([64, 512], F32, tag="bcB", name="bcB")
            nc.gpsimd.partition_broadcast(bcB[:], recB[:])
            nc.vector.tensor_tensor(
                yT[fo][0:64, q0 : q0 + 512], py[0:64, 0:512], bcA[:], op=ALU.mult
            )
            nc.vector.tensor_tensor(
                yT[fo][64:128, q0 : q0 + 512], py[0:64, 512:1024], bcB[:], op=ALU.mult
            )

        def sexp_attn(fo, b, after_first=None):
            if b == 0:
                return sexp_attn0(fo, after_first)
            q0 = 512 * b
            # chunk pairs in A@V DoubleRow j-dim.  diag pair 0 first (its
            # full-width A@V start matmul zeroes the psum region); then
            # interleave full pairs (ACT exp) with diag pair 1 (DVE exp) so
            # neither engine starves during the unit.
            fulls = [(2 * j, 0) for j in range(2 * b)]
            pairs = [(4 * b, 0)] + fulls[0:1] + [(4 * b + 2, 256)] + fulls[1:]
            ests = []
            for pi, (c0, av_qoff) in enumerate(pairs):
                eST = p2e.tile([128, 2, 1024], U8, tag="eST", name="eST")
                diag = c0 >= 4 * b
                if diag:
                    # zero the columns of this pair that no chunk writes
                    i0 = c0 - 4 * b
                    if i0 == 0:
                        nc.gpsimd.memset(
                            eST[:, 1, :].rearrange("p (h w) -> p h w", h=2)[:, :, 0:128],
                            0,
                        )
                    else:
                        nc.gpsimd.memset(
                            eST[:, 1, :].rearrange("p (h w) -> p h w", h=2)[:, :, 256:384],
                            0,
                        )
                for c in (c0, c0 + 1):
                    i = c - 4 * b
                    qoff = 128 * i if diag else 0
                    w = 512 - qoff
                    pS = psS.tile([128, 1024], F32, tag="pS", name="pS")
                    for hb in (0, 64):
                        nc.tensor.matmul(
                            pS[:, (0 if hb == 0 else 512) + qoff : (512 if hb == 0 else 1024)],
                            _j2(kT8[fo][hb : hb + 64, 128 * c : 128 * (c + 1)]).bitcast(F8E4),
                            _j2(qT8[fo][hb : hb + 64, q0 + qoff : q0 + 512]).bitcast(F8E4),
                            perf_mode=DR,
                        )
                    if diag:
                        # fused exp2-bits + causal mask + fp8 quantize (DVE)
                        nc.vector.scalar_tensor_tensor(
                            eST[:, c % 2, :].rearrange("p (h w) -> p h w", h=2)[:, :, qoff:512],
                            pS[:].rearrange("p (h w) -> p h w", h=2)[:, :, qoff:512],
                            K8,
                            masks_sb[:, :, 0:w],
                            op0=ALU.mult,
                            op1=ALU.add,
                        )
                    else:
                        # full chunk: true exp on ACT; in the ACT-hot rows a
                        # fraction goes to DVE instead
                        if (b == 3 and full_ctr[0] % 6 == 2) or (
                            b == 2 and full_ctr[0] % 6 == 3
                        ):
                            nc.vector.tensor_scalar(
                                eST[:, c % 2, :], pS[:], K8, EXPB, op0=ALU.mult, op1=ALU.add
                            )
                        else:
                            nc.scalar.activation(
                                eST[:, c % 2, :].bitcast(F8E4), pS[:], AF.Exp, scale=ACT_SCALE
                            )
                        full_ctr[0] += 1
                ests.append((c0, av_qoff, eST))
                if pi == 0 and after_first is not None:
                    after_first()
            return ests

        def av_attn(fo, b, ests):
            if b == 0:
                av_attn0(fo, ests)
                return
            py = psY.tile([128, 1024], F32, tag="py", name="py")
            for pi, (c0, av_qoff, eST) in enumerate(ests):
                first = pi == 0
                last = pi == len(ests) - 1
                for hb, h in ((0, 2 * fo), (64, 2 * fo + 1)):
                    col = 0 if hb == 0 else 512
                    nc.tensor.matmul(
                        py[0:65, col + av_qoff : col + 512],
                        v8[c0 // 2][:, :, h, 0:65].bitcast(F8E4),
                        eST[:, :, col + av_qoff : col + 512].bitcast(F8E4),
                        start=first,
                        stop=last,
                        perf_mode=DR,
                    )
            emit_norm(fo, b, py)

        for g in range(12):
            emit_qkv_group(0, g)
        from collections import deque

        pend = deque()
        for b in range(NTQ):
            for fo in range(NFO):
                cb = (lambda: av_attn(*pend.popleft())) if pend else None
                st = sexp_attn(fo, b, after_first=cb)
                pend.append((fo, b, st))
                if b < NTQ - 1:
                    for g in range(3 * fo, 3 * fo + 3):
                        emit_qkv_group(b + 1, g)
                if b == 1:
                    emit_proj_tc(fo)
                elif b == 3:
                    emit_proj_tc(4 + fo)
                    emit_proj_tc(8 + fo)
        while pend:
            av_attn(*pend.popleft())
        for tci in range(12, 16):
            emit_proj_tc(tci)

    nc.compile()
    return nc


def _get_nc():
    if "nc" not in _cached:
        _cached["nc"] = _build_nc()
    return _cached["nc"]


def _e4m3(a):
    return np.asarray(a, np.float32).astype(ml_dtypes.float8_e4m3).view(np.uint8)


def kernel(x, W_attn, b_attn, W_proj, b_proj):
    x = np.asarray(x, np.float32)
    W_attn = np.asarray(W_attn, np.float32)
    b_attn = np.asarray(b_attn, np.float32)
    W_proj = np.asarray(W_proj, np.float32)
    b_proj = np.asarray(b_proj, np.float32)

    nc = _get_nc()
    p = np.arange(128)[:, None]
    j = np.arange(128)[None, :]
    tri = np.where(j >= p, EXPB, -30000.0).astype(np.float32)   # [128,128]
    mask1 = np.concatenate([tri, np.full((128, 384), EXPB, np.float32)], axis=1)
    masks = np.stack([mask1, mask1], axis=1).astype(ml_dtypes.bfloat16)  # [128,2,512]

    ones8 = np.zeros((128, 2, 512), np.uint8)
    ones8[0, 0, :] = _e4m3(1.0)
    tri01 = (j >= p).astype(np.float32)
    m01 = np.stack([tri01, tri01], axis=1).astype(ml_dtypes.bfloat16)  # [128,2,128]

    in_maps = []
    for c in range(N_CORES):
        b, hh = divmod(c, 2)
        sl = slice(CH * hh, CH * (hh + 1))
        bqk = np.zeros((128, 2, 1024), np.uint8)
        bqk[0, 0, 0:512] = _e4m3(WSC * b_attn[0:C][sl])
        bqk[0, 0, 512:1024] = _e4m3(WSC * b_attn[C : 2 * C][sl])
        in_maps.append(
            {
                "x8": _e4m3(np.ascontiguousarray(x[b].T)),
                "wq8": _e4m3(WSC * W_attn[:, 0:C][:, sl]),
                "wk8": _e4m3(WSC * W_attn[:, C : 2 * C][:, sl]),
                "wv8": _e4m3(WSC * W_attn[:, 2 * C : 3 * C][:, sl]),
                "bqk8": bqk,
                "ones8": ones8,
                "masks": masks.view(np.uint16),
                "wp": np.ascontiguousarray(
                    W_proj[sl, :].astype(ml_dtypes.bfloat16)
                ).view(np.uint16),
                "xb": np.ascontiguousarray(x[b].T[:, 0:512]).astype(ml_dtypes.bfloat16).view(np.uint16),
                "wvb": np.ascontiguousarray(WSC * W_attn[:, 2 * C : 3 * C][:, sl]).astype(ml_dtypes.bfloat16).view(np.uint16),
                "m01": m01.view(np.uint16),
            }
        )

    try:
        res = run_bass_kernel_spmd(nc, in_maps, core_ids=list(range(N_CORES)))
    except Exception:
        # transient NRT device wedges happen; one retry is usually enough
        res = run_bass_kernel_spmd(nc, in_maps, core_ids=list(range(N_CORES)))

    bv = b_attn[2 * C : 3 * C]
    const_bias = (bv @ W_proj + b_proj).astype(np.float32)  # [C]
    def _as_f32(a):
        a = np.asarray(a)
        if a.dtype == np.uint16:
            a = a.view(ml_dtypes.bfloat16)
        return a.astype(np.float32)

    out = np.empty((B, T, C), np.float32)
    for b in range(B):
        out[b] = (
            _as_f32(res.results[2 * b]["out"])
            + _as_f32(res.results[2 * b + 1]["out"])
            + const_bias
        )
    return out
